# revision 14
# baseline (speedup 1.0000x reference)
"""MiniRocketFeaturesPlus Trainium2 kernel (v4).

Strategy (data-parallel over batch, 4 samples per core on 8 cores):
  - Tap-shifted slab: xm[(t*9+c), b*2048 + l] = x_pad[c, b, PAD - 4d + d*t + l]
    for t = 0..12 (117 contraction rows).  With p = 4d, the cropped half's
    window [p, p+V) is reachable from the SAME rhs columns via tap index
    t+4, so one matmul per (dilation, sample) emits all 84 kernel rows.
  - 64/20 chunking into 3 PSUM tiles per dilation (4 samples):
      tile A = [s0 rows 0:64 @p0, s1 rows 0:64 @p64]   width 2048
      tile B = [s2 rows 0:64 @p0, s3 rows 0:64 @p64]   width 2048
      tile C = [s0 rows 64:84 @p0, s1 @p32, s2 @p64, s3 @p96] width V
  - PSUM tiles staged to SBUF fp16 by ACT (scalar.copy); h1-row tails of
    A/B zeroed by Pool memsets.  PPV reductions on the fp16 SBUF tiles:
      * cnt features use the PAIR trick: DVE builds J0 = (C_A > t) * 4096
        with a 4x-mode TENSOR_SCALAR (no accum), then one 1x
        SCALAR_TENSOR_TENSOR accumulation (C_B > t) + J0 yields
        acc = N_B + 4096 * N_A exactly (fp32-internal accumulator) -
        one 1x accumulation covers BOTH tiles.  C-tile cnt pairs two
        features the same way.
      * rel features: max(C, t) accum on DVE / Relu(C - t) accum on ACT,
        greedily balanced.
  - fp32 dils (24, 25) keep hi+lo double matmuls and direct-PSUM passes.
  - Decode on host: a = N/L, b = R / max(2R - (S - L t), 1e-8), with S
    computed on the host from prefix sums.
"""

import numpy as np
import sys

sys.path.insert(0, "/opt/trn_rl_repo")

C_IN, SEQ_LEN = 9, 2048
KERNEL_SIZE = 9
NUM_KERNELS = 84
B = 32
N_CORES = 8
B_CORE = B // N_CORES
PAD = 1020
NTAP = 13
KDIM = NTAP * C_IN  # 117
LP2 = PAD + 8 * 255 + SEQ_LEN + 4  # 5111: max read PAD+8d+2047
XCOLS = B_CORE * SEQ_LEN  # 8192
FP32_DILS = (24, 25)
PAIR_W = 4096.0

# per-pass cost model (ns) - HW-measured on TRN2
def _c_ts2ap(w):   # 4x TENSOR_SCALAR, 2 AP scalars, no accum (J0 build)
    return w * 0.26 + 280
def _c_stt(w):     # SCALAR_TENSOR_TENSOR with accum (1x)
    return w * 1.0417 + 219
def _c_dve_cr(w):  # TENSOR_SCALAR_CACHE_REDUCE fp16 SBUF (1x)
    return w * 1.0417 + 125
def _c_dve32p(w):  # 1x, fp32 PSUM
    return w * 1.0417 + 170
def _c_act16(w):   # ACTIVATE fp16 SBUF + accum read
    return w * 0.8333 + 480
def _c_act32p(w):  # ACTIVATE fp32 PSUM + accum read
    return w * 0.8333 + 550
def _c_act_stage(w):
    return w * 0.8333 + 219
def _c_pool_memset(w):
    return w * 0.8333 + 131


def _config():
    nf_total = 10000 // 2 // NUM_KERNELS * NUM_KERNELS
    nfpk = nf_total // NUM_KERNELS
    true_max = min(nfpk, 32)
    multiplier = nfpk / true_max
    max_exp = np.log2((SEQ_LEN - 1) / (KERNEL_SIZE - 1))
    dilations, counts = np.unique(
        np.logspace(0, max_exp, true_max, base=2).astype(np.int32),
        return_counts=True)
    nfpd = (counts * multiplier).astype(np.int32)
    rem = nfpk - nfpd.sum()
    i = 0
    while rem > 0:
        nfpd[i] += 1
        rem -= 1
        i = (i + 1) % len(nfpd)
    paddings = [(KERNEL_SIZE - 1) * int(d) // 2 for d in dilations]
    return [int(d) for d in dilations], paddings, [int(n) for n in nfpd]


DILS, PADS, NFPD = _config()
ND = len(DILS)
DIL_ORDER = [0, 24, 21, 1, 25, 22, 4, 23] + [i for i in range(ND)
                                             if i not in (0, 1, 4, 21, 22,
                                                          23, 24, 25)]


def _halves(i):
    p1 = i % 2
    h0 = list(range(p1, NUM_KERNELS, 2))
    h1 = list(range(1 - p1, NUM_KERNELS, 2))
    return h0, h1


class Plan4:
    """Static schedule: stages, passes, engine assignment, decode maps."""

    def __init__(self):
        self.passes = []
        load = {"dve": 0.0, "act": 0.0, "pool": 0.0}
        ncol = {"dve": 0, "act": 0, "pool": 0}
        self._nthr = 0

        def new_thr():
            self._nthr += 1
            return self._nthr - 1

        def new_acc(eng):
            ncol[eng] += 1
            return ncol[eng] - 1

        self.stage_eng = {}
        for i in DIL_ORDER:
            nf = NFPD[i]
            V = SEQ_LEN - 2 * PADS[i]
            # staging: balance between ACT (scalar.copy) and DVE (tensor_copy)
            for tile in ("A", "B", "C"):
                w = SEQ_LEN if tile in ("A", "B") else V
                ca = _c_act_stage(w)
                cd = w * 1.0417 + 170  # DVE copy from fp32 PSUM, 1x
                eng = "act" if load["act"] + ca <= load["dve"] + cd else "dve"
                load[eng] += ca if eng == "act" else cd
                self.stage_eng[(i, tile)] = eng
            load["pool"] += 4 * _c_pool_memset(SEQ_LEN - V)
            # cnt pairs across A/B per feature (DVE only)
            for f in range(nf):
                load["dve"] += _c_ts2ap(SEQ_LEN) + _c_stt(SEQ_LEN)
                self.passes.append(dict(
                    kind="cntpair", dil=i, f=f, eng="dve",
                    thr=new_thr(), acc=new_acc("dve"), w=SEQ_LEN,
                    staged=True))
            # C-tile cnt: feature pairs + odd single
            f = 0
            while f + 1 < nf:
                load["dve"] += _c_ts2ap(V) + _c_stt(V)
                self.passes.append(dict(
                    kind="cntpairC", dil=i, f=f, f1=f + 1, eng="dve",
                    thr=new_thr(), thr1=new_thr(), acc=new_acc("dve"),
                    w=V, staged=True))
                f += 2
            if f < nf:
                cd = _c_dve_cr(V)
                ca = _c_act16(V)
                eng = "dve" if load["dve"] + cd <= load["act"] + ca else "act"
                load[eng] += cd if eng == "dve" else ca
                self.passes.append(dict(
                    kind="cnt", dil=i, tile="C", f=f, eng=eng,
                    thr=new_thr(), acc=new_acc(eng), w=V, staged=True))
            # rel passes: balance dve/act
            for f in range(nf):
                for tile in ("A", "B", "C"):
                    w = SEQ_LEN if tile in ("A", "B") else V
                    cd = _c_dve_cr(w)
                    ca = _c_act16(w)
                    eng = ("dve" if load["dve"] + cd <= load["act"] + ca
                           else "act")
                    load[eng] += cd if eng == "dve" else ca
                    self.passes.append(dict(
                        kind="rel", dil=i, tile=tile, f=f, eng=eng,
                        thr=new_thr(), acc=new_acc(eng), w=w, staged=True))
        self.nthr = self._nthr
        self.ncol = dict(ncol)
        self.est = dict(load)
        self.off = np.concatenate([[0], np.cumsum(NFPD)]).astype(int)

    # --- partition layout -------------------------------------------------
    # tile A/B partition p: sample = sA + (p >= 64); r = p % 64
    #   r in [0,22): h1[r]; r in [22,64): h0[r-22]
    # tile C partition p: sample = p // 32; r = p % 32 (valid r < 20): h1[22+r]
    def _t_of(self, i, f, tile):
        h0, h1 = _halves(i)
        t_of = np.zeros(128, np.float32)
        use = np.zeros(128, bool)
        biases = self._biases
        if tile in ("A", "B"):
            for half in range(2):
                for r in range(22):
                    t_of[half * 64 + r] = biases[h1[r], self.off[i] + f]
                    use[half * 64 + r] = True
                for r in range(42):
                    t_of[half * 64 + 22 + r] = biases[h0[r], self.off[i] + f]
                    use[half * 64 + 22 + r] = True
        else:
            for s in range(4):
                for r in range(20):
                    t_of[s * 32 + r] = biases[h1[22 + r], self.off[i] + f]
                    use[s * 32 + r] = True
        return t_of, use

    def build_thresholds(self, biases):
        self._biases = np.asarray(biases, np.float32)
        thr = np.zeros((128, self.nthr), np.float32)
        for pa in self.passes:
            i, f, eng = pa["dil"], pa["f"], pa["eng"]
            kind = pa["kind"]
            if kind == "cntpair":
                t, use = self._t_of(i, f, "A")
                col = t.copy()
                col[~use] = 0.0
                thr[:, pa["thr"]] = col
            elif kind == "cntpairC":
                for fx, tkey in ((pa["f"], "thr"), (pa["f1"], "thr1")):
                    t, use = self._t_of(i, fx, "C")
                    col = t.copy()
                    col[~use] = 0.0
                    thr[:, pa[tkey]] = col
            elif kind == "cnt":
                t, use = self._t_of(i, f, pa["tile"])
                if eng == "dve":
                    col = t.copy()  # is_gt(c, t)
                else:
                    tp = np.nextafter(t, np.float32(np.inf)).astype(np.float32)
                    col = -tp  # Sign(c - t')
                col[~use] = 0.0
                thr[:, pa["thr"]] = col
            else:  # rel
                t, use = self._t_of(i, f, pa["tile"])
                col = t.copy() if eng == "dve" else -t  # max(c,t) / Relu(c-t)
                col[~use] = 0.0
                thr[:, pa["thr"]] = col
        return thr

    # --- host-side S (exact window sums of bf16-cast data) ----------------
    def host_S(self, x_pad_bf, w2t):
        P = np.zeros((B, C_IN, LP2 + 1), np.float64)
        np.cumsum(x_pad_bf.astype(np.float64), axis=2, out=P[:, :, 1:])
        S = {}
        for i, (d, p, nf) in enumerate(zip(DILS, PADS, NFPD)):
            V = SEQ_LEN - 2 * p
            o0 = np.array([PAD - 4 * d + d * t for t in range(NTAP)])
            s_arr = np.zeros((2, 42, B), np.float64)
            w = w2t[i]
            xs0 = np.zeros((KDIM, B), np.float64)
            xs1 = np.zeros((KDIM, B), np.float64)
            for t in range(NTAP):
                o = o0[t]
                for c in range(C_IN):
                    xs0[t * 9 + c] = P[:, c, o + SEQ_LEN] - P[:, c, o]
                    xs1[t * 9 + c] = P[:, c, o + V] - P[:, c, o]
            s_arr[0] = w[:, 22:64].T @ xs0
            s_arr[1, 0:22] = w[:, 0:22].T @ xs1
            s_arr[1, 22:42] = w[:, 64:84].T @ xs1
            S[i] = s_arr
        return S

    # --- decode -----------------------------------------------------------
    def decode(self, accs, biases, S):
        """accs: eng -> [ncores, 128, ncol_eng] -> [B, 9912] fp32."""
        biases = np.asarray(biases, np.float64)
        blk_off = np.concatenate(
            [[0], np.cumsum([168 * nf for nf in NFPD])]).astype(int)
        out = np.zeros((B, blk_off[-1]), np.float64)
        N = {}
        R = {}

        def tgt_of(i, f, typ):
            key = (i, f)
            if key not in N:
                N[key] = np.full((2, 42, N_CORES, B_CORE), np.nan)
                R[key] = np.full((2, 42, N_CORES, B_CORE), np.nan)
            return N[key] if typ == "cnt" else R[key]

        def apply_AB(i, f, tile, typ, is_dve, v):
            """v: [ncores, 128] raw accum for an A/B tile scan."""
            V = SEQ_LEN - 2 * PADS[i]
            h0, h1 = _halves(i)
            tgt = tgt_of(i, f, typ)
            sbase = 0 if tile == "A" else 2
            for half in range(2):
                s = sbase + half
                vv = v[:, half * 64: half * 64 + 64]
                raw1 = vv[:, 0:22]
                raw0 = vv[:, 22:64]
                t0 = biases[h0, self.off[i] + f][None, :]
                t1 = biases[h1[0:22], self.off[i] + f][None, :]
                T2 = SEQ_LEN - V
                if typ == "cnt":
                    if is_dve:
                        n0 = raw0
                        n1 = raw1 - T2 * (t1 < 0)
                    else:
                        n0 = (raw0 + SEQ_LEN) / 2
                        tp1 = np.nextafter(
                            t1.astype(np.float32), np.float32(np.inf)
                        ).astype(np.float64)
                        n1 = (raw1 + T2 * np.sign(tp1) + V) / 2
                    tgt[0, :, :, s] = n0.T.reshape(42, N_CORES)
                    tgt[1, 0:22, :, s] = n1.T.reshape(22, N_CORES)
                else:
                    if is_dve:
                        r0 = raw0 - SEQ_LEN * t0
                        r1 = raw1 - T2 * np.maximum(t1, 0) - V * t1
                    else:
                        r0 = raw0
                        r1 = raw1 - T2 * np.maximum(-t1, 0)
                    tgt[0, :, :, s] = r0.T.reshape(42, N_CORES)
                    tgt[1, 0:22, :, s] = r1.T.reshape(22, N_CORES)

        def apply_C(i, f, typ, is_dve, v):
            V = SEQ_LEN - 2 * PADS[i]
            h0, h1 = _halves(i)
            tgt = tgt_of(i, f, typ)
            t1 = biases[h1[22:42], self.off[i] + f][None, :]
            for s in range(4):
                vv = v[:, s * 32: s * 32 + 20]
                if typ == "cnt":
                    n1 = vv if is_dve else (vv + V) / 2
                    tgt[1, 22:42, :, s] = n1.T.reshape(20, N_CORES)
                else:
                    r1 = vv - V * t1 if is_dve else vv
                    tgt[1, 22:42, :, s] = r1.T.reshape(20, N_CORES)

        for pa in self.passes:
            i, f, eng, kind = pa["dil"], pa["f"], pa["eng"], pa["kind"]
            acc = accs[eng][:, :, pa["acc"]]  # [ncores, 128]
            if kind == "cntpair":
                nA = np.floor(acc / PAIR_W + 0.5 / PAIR_W)
                nB = acc - PAIR_W * nA
                apply_AB(i, f, "A", "cnt", True, nA)
                apply_AB(i, f, "B", "cnt", True, nB)
            elif kind == "cntpairC":
                n0 = np.floor(acc / PAIR_W + 0.5 / PAIR_W)
                n1 = acc - PAIR_W * n0
                apply_C(i, pa["f"], "cnt", True, n0)
                apply_C(i, pa["f1"], "cnt", True, n1)
            elif kind in ("cnt", "rel"):
                typ = kind
                is_dve = eng == "dve"
                if pa["tile"] in ("A", "B"):
                    apply_AB(i, f, pa["tile"], typ, is_dve, acc)
                else:
                    apply_C(i, f, typ, is_dve, acc)
        # assemble features
        for i, (d, p, nf) in enumerate(zip(DILS, PADS, NFPD)):
            V = SEQ_LEN - 2 * p
            h0, h1 = _halves(i)
            base = blk_off[i]
            s_arr = S[i]
            for f in range(nf):
                t0 = biases[h0, self.off[i] + f]
                t1 = biases[h1, self.off[i] + f]
                n = N[(i, f)]
                r = R[(i, f)]
                for core in range(N_CORES):
                    for bb in range(B_CORE):
                        gb = core * B_CORE + bb
                        n0 = n[0, :, core, bb]
                        n1 = n[1, :, core, bb]
                        r0 = r[0, :, core, bb]
                        r1 = r[1, :, core, bb]
                        s0 = s_arr[0, :, gb]
                        s1 = s_arr[1, :, gb]
                        a0 = n0 / SEQ_LEN
                        a1 = n1 / V
                        d0 = s0 - SEQ_LEN * t0
                        d1 = s1 - V * t1
                        b0 = r0 / np.maximum(2 * r0 - d0, 1e-8)
                        b1 = r1 / np.maximum(2 * r1 - d1, 1e-8)
                        idx = np.arange(42)
                        out[gb, base + f + nf * idx] = a0
                        out[gb, base + 42 * nf + f + nf * idx] = b0
                        out[gb, base + 84 * nf + f + nf * idx] = a1
                        out[gb, base + 126 * nf + f + nf * idx] = b1
        return out.astype(np.float32)


PLAN = Plan4()
_NC_CACHE = {}


def _build_nc():
    import concourse.bacc as bacc
    import concourse.tile as tile
    from concourse import mybir
    import concourse.bass as bass

    nc = bacc.Bacc()
    x_pad = nc.dram_tensor("x_pad", [C_IN, B_CORE, LP2], mybir.dt.bfloat16,
                           kind="ExternalInput")
    x_pad_lo = nc.dram_tensor("x_pad_lo", [C_IN, B_CORE, LP2],
                              mybir.dt.bfloat16, kind="ExternalInput")
    w2 = nc.dram_tensor("w2", [ND, KDIM, 84], mybir.dt.bfloat16,
                        kind="ExternalInput")
    thr_in = nc.dram_tensor("thr_in", [128, PLAN.nthr], mybir.dt.float32,
                            kind="ExternalInput")
    acc_out = {}
    for eng in ("dve", "act"):
        if PLAN.ncol[eng]:
            acc_out[eng] = nc.dram_tensor(
                f"acc_{eng}", [128, PLAN.ncol[eng]], mybir.dt.float32,
                kind="ExternalOutput")

    from collections import defaultdict
    dil_passes = defaultdict(list)
    for pa in PLAN.passes:
        dil_passes[pa["dil"]].append(pa)

    gt = mybir.AluOpType.is_gt
    mx = mybir.AluOpType.max
    add = mybir.AluOpType.add
    mult = mybir.AluOpType.mult

    with tile.TileContext(nc) as tc:
        with tc.tile_pool(name="sb", bufs=1) as sb, \
             tc.tile_pool(name="slab", bufs=2) as slab_pool, \
             tc.tile_pool(name="st", bufs=8) as st_pool, \
             tc.tile_pool(name="ps", bufs=2, space="PSUM") as ps:
            w2_sb = sb.tile([KDIM, ND, 84], mybir.dt.bfloat16, tag="w2")
            thr = sb.tile([128, PLAN.nthr], mybir.dt.float32, tag="thr")
            w4096 = sb.tile([128, 1], mybir.dt.float32, tag="w4096")
            j0 = sb.tile([128, SEQ_LEN], mybir.dt.float16, tag="j0")
            acc_sb = {}
            junk = {}
            for eng in ("dve", "act"):
                if PLAN.ncol[eng]:
                    acc_sb[eng] = sb.tile([128, PLAN.ncol[eng]],
                                          mybir.dt.float32, tag=f"acc_{eng}",
                                          name=f"acc_{eng}")
                junk[eng] = sb.tile([128, SEQ_LEN], mybir.dt.float16,
                                    tag=f"junk_{eng}", name=f"junk_{eng}")

            nc.sync.dma_start(out=w2_sb, in_=w2[:, :, :].transpose([1, 0, 2]))
            nc.sync.dma_start(out=thr, in_=thr_in[:, :])
            nc.vector.memset(w4096, PAIR_W)

            def emit_pass(pa, tiles, V):
                eng = pa["eng"]
                kind = pa["kind"]
                w = pa["w"]
                tcol = thr[:, pa["thr"]:pa["thr"] + 1]
                if kind == "cntpair":
                    nc.vector.tensor_scalar(
                        out=j0, in0=tiles["A"], scalar1=tcol, scalar2=w4096,
                        op0=gt, op1=mult)
                    nc.vector.scalar_tensor_tensor(
                        out=junk["dve"], in0=tiles["B"], scalar=tcol,
                        in1=j0, op0=gt, op1=add,
                        accum_out=acc_sb["dve"][:, pa["acc"]:pa["acc"] + 1])
                    return
                if kind == "cntpairC":
                    t1col = thr[:, pa["thr1"]:pa["thr1"] + 1]
                    nc.vector.tensor_scalar(
                        out=j0[:, 0:w], in0=tiles["C"][:, 0:w], scalar1=tcol,
                        scalar2=w4096, op0=gt, op1=mult)
                    nc.vector.scalar_tensor_tensor(
                        out=junk["dve"][:, 0:w], in0=tiles["C"][:, 0:w],
                        scalar=t1col, in1=j0[:, 0:w], op0=gt, op1=add,
                        accum_out=acc_sb["dve"][:, pa["acc"]:pa["acc"] + 1])
                    return
                src = tiles[pa["tile"]]
                acol = acc_sb[eng][:, pa["acc"]:pa["acc"] + 1]
                if eng == "act":
                    func = (mybir.ActivationFunctionType.Sign
                            if kind == "cnt"
                            else mybir.ActivationFunctionType.Relu)
                    nc.scalar.activation(
                        out=junk["act"][:, 0:w], in_=src[:, 0:w],
                        func=func, bias=tcol, scale=1.0, accum_out=acol)
                else:
                    op0 = gt if kind == "cnt" else mx
                    nc.vector.tensor_scalar(
                        out=junk["dve"][:, 0:w], in0=src[:, 0:w],
                        scalar1=tcol, scalar2=None, op0=op0,
                        op1=add, accum_out=acol)

            for i in DIL_ORDER:
                d, p, nf = DILS[i], PADS[i], NFPD[i]
                V = SEQ_LEN - 2 * p
                is32 = i in FP32_DILS

                def _gather(dst_tile, src_dram):
                    full = src_dram[:, :, :]
                    for c in range(C_IN):
                        src_ap = bass.AP(
                            tensor=full.tensor,
                            offset=PAD - 4 * d + c * B_CORE * LP2,
                            ap=[[d, NTAP], [LP2, B_CORE], [1, SEQ_LEN]],
                        )
                        dst_ap = bass.AP(
                            tensor=dst_tile.tensor,
                            offset=dst_tile.offset + c * XCOLS,
                            ap=[[C_IN * XCOLS, NTAP], [SEQ_LEN, B_CORE],
                                [1, SEQ_LEN]],
                        )
                        nc.sync.dma_start(out=dst_ap, in_=src_ap)

                xm = slab_pool.tile([KDIM, XCOLS], mybir.dt.bfloat16, tag="xm")
                _gather(xm, x_pad)
                xm_lo = None
                if is32:
                    xm_lo = slab_pool.tile([KDIM, XCOLS], mybir.dt.bfloat16,
                                           tag="xmlo")
                    _gather(xm_lo, x_pad_lo)

                lhs1 = w2_sb[:, i, 0:64]
                lhs2 = w2_sb[:, i, 64:84]

                def _stage(tname, pt, w):
                    st = st_pool.tile([128, SEQ_LEN], mybir.dt.float16,
                                      tag="st")
                    if PLAN.stage_eng[(i, tname)] == "act":
                        nc.scalar.copy(out=st[:, 0:w], in_=pt[:, 0:w])
                    else:
                        nc.vector.tensor_copy(out=st[:, 0:w], in_=pt[:, 0:w])
                    return st

                tiles = {}
                for tname, samples in (("A", (0, 1)), ("B", (2, 3))):
                    pt = ps.tile([128, SEQ_LEN], mybir.dt.float32, tag="pt")
                    for si, s in enumerate(samples):
                        x0 = s * SEQ_LEN
                        pb = si * 64
                        for k in range(4):
                            o = pt[pb:pb + 64, 512 * k:512 * (k + 1)]
                            nc.tensor.matmul(
                                o, lhs1,
                                xm[:, x0 + 512 * k: x0 + 512 * (k + 1)],
                                start=True, stop=not is32,
                                tile_position=(0, pb))
                            if is32:
                                nc.tensor.matmul(
                                    o, lhs1,
                                    xm_lo[:, x0 + 512 * k: x0 + 512 * (k + 1)],
                                    start=False, stop=True,
                                    tile_position=(0, pb))
                    st = _stage(tname, pt, SEQ_LEN)
                    nc.gpsimd.memset(st[0:22, V:SEQ_LEN], 0.0)
                    nc.gpsimd.memset(st[64:86, V:SEQ_LEN], 0.0)
                    tiles[tname] = st

                # tile C: rows 64:84 of each sample, width V
                pt = ps.tile([128, SEQ_LEN], mybir.dt.float32, tag="pt")
                for s in range(4):
                    x0 = s * SEQ_LEN
                    pb = s * 32
                    kv, rem = divmod(V, 512)
                    chunks = [(512 * k, 512) for k in range(kv)]
                    if rem:
                        chunks.append((512 * kv, rem))
                    for co, cw in chunks:
                        o = pt[pb:pb + 20, co:co + cw]
                        nc.tensor.matmul(
                            o, lhs2, xm[:, x0 + co: x0 + co + cw],
                            start=True, stop=not is32,
                            tile_position=(0, pb))
                        if is32:
                            nc.tensor.matmul(
                                o, lhs2, xm_lo[:, x0 + co: x0 + co + cw],
                                start=False, stop=True,
                                tile_position=(0, pb))
                tiles["C"] = _stage("C", pt, V)

                for pa in dil_passes[i]:
                    emit_pass(pa, tiles, V)

            for eng, t in acc_sb.items():
                nc.sync.dma_start(out=acc_out[eng][:, :], in_=t)
    nc.compile()
    return nc


def _host_prep(x, kernels, channel_combinations):
    x = np.asarray(x, np.float32)
    kernels = np.asarray(kernels, np.float32)
    cc = np.asarray(channel_combinations, np.float32)
    x_pad = np.zeros((B, C_IN, LP2), np.float32)
    x_pad[:, :, PAD:PAD + SEQ_LEN] = x
    kern = kernels.reshape(C_IN, NUM_KERNELS, KERNEL_SIZE)
    w2t = np.zeros((ND, KDIM, 84), np.float32)
    for i in range(ND):
        h0, h1 = _halves(i)
        for t in range(KERNEL_SIZE):
            for c in range(C_IN):
                w2t[i, (t + 4) * 9 + c, 0:22] = cc[i, c, h1[0:22]] * kern[c, h1[0:22], t]
                w2t[i, t * 9 + c, 22:64] = cc[i, c, h0] * kern[c, h0, t]
                w2t[i, (t + 4) * 9 + c, 64:84] = cc[i, c, h1[22:42]] * kern[c, h1[22:42], t]
    return x_pad, w2t


def _make_in_maps(x_pad, w2t, thr, cores):
    import ml_dtypes
    w2bf = w2t.astype(ml_dtypes.bfloat16)
    in_maps = []
    for core in cores:
        xs = np.ascontiguousarray(
            x_pad[core * B_CORE:(core + 1) * B_CORE].transpose(1, 0, 2))
        xhi = xs.astype(ml_dtypes.bfloat16)
        xlo = (xs - xhi.astype(np.float32)).astype(ml_dtypes.bfloat16)
        in_maps.append({
            "x_pad": xhi,
            "x_pad_lo": xlo,
            "w2": w2bf,
            "thr_in": thr,
        })
    return in_maps


def kernel(x, kernels, channel_combinations, biases, _run_cores=None):
    from concourse.bass_utils import run_bass_kernel_spmd

    x_pad, w2t = _host_prep(x, kernels, channel_combinations)
    thr = PLAN.build_thresholds(np.asarray(biases, np.float32))

    if "nc" not in _NC_CACHE:
        _NC_CACHE["nc"] = _build_nc()
    nc = _NC_CACHE["nc"]

    cores = list(range(N_CORES)) if _run_cores is None else _run_cores
    in_maps = _make_in_maps(x_pad, w2t, thr, cores)
    res = run_bass_kernel_spmd(nc, in_maps, core_ids=cores)
    accs = {}
    for eng in ("dve", "act", "pool"):
        if PLAN.ncol[eng]:
            accs[eng] = np.stack([r[f"acc_{eng}"] for r in res.results])
            if _run_cores is not None:
                reps = N_CORES // len(cores)
                accs[eng] = np.concatenate([accs[eng]] * reps)
        else:
            accs[eng] = np.zeros((N_CORES, 128, 0), np.float32)
    S = _host_S_all(x_pad, w2t)
    return PLAN.decode(accs, biases, S)


def _x_eff(x_pad):
    import ml_dtypes
    xhi = x_pad.astype(ml_dtypes.bfloat16).astype(np.float32)
    xlo = (x_pad - xhi).astype(ml_dtypes.bfloat16).astype(np.float32)
    return xhi, xhi + xlo


def _host_S_all(x_pad, w2t):
    import ml_dtypes
    xhi, xhilo = _x_eff(x_pad)
    wb = w2t.astype(ml_dtypes.bfloat16).astype(np.float64)
    S = {}
    S_bf = PLAN.host_S(xhi.astype(np.float64), wb)
    S_32 = None
    for i in range(ND):
        if i in FP32_DILS:
            if S_32 is None:
                S_32 = PLAN.host_S(xhilo.astype(np.float64), wb)
            S[i] = S_32[i]
        else:
            S[i] = S_bf[i]
    return S


def sim_accums(x, kernels, channel_combinations, biases):
    """Numpy simulation of device accumulators (decode validation)."""
    import ml_dtypes
    x_pad, w2t = _host_prep(x, kernels, channel_combinations)
    thr = PLAN.build_thresholds(np.asarray(biases, np.float32))
    xhi, xhilo = _x_eff(x_pad)
    wb = w2t.astype(ml_dtypes.bfloat16).astype(np.float32)
    accs = {eng: np.zeros((N_CORES, 128, PLAN.ncol[eng]), np.float32)
            for eng in ("dve", "act", "pool")}
    for core in range(N_CORES):
        Ctiles = {}
        for i, (d, p, nf) in enumerate(zip(DILS, PADS, NFPD)):
            V = SEQ_LEN - 2 * p
            is32 = i in FP32_DILS
            xs = (xhilo if is32 else xhi)[core * B_CORE:(core + 1) * B_CORE]
            w = wb[i]
            C84 = np.zeros((4, 84, SEQ_LEN), np.float32)
            for s in range(B_CORE):
                xsl = np.zeros((KDIM, SEQ_LEN), np.float32)
                for t in range(NTAP):
                    o = PAD - 4 * d + d * t
                    for c in range(C_IN):
                        xsl[t * 9 + c] = xs[s, c, o:o + SEQ_LEN]
                C84[s] = w.T @ xsl
            C84 = C84.astype(np.float16).astype(np.float32)
            tA = np.zeros((128, SEQ_LEN), np.float32)
            tB = np.zeros((128, SEQ_LEN), np.float32)
            tC = np.zeros((128, SEQ_LEN), np.float32)
            for si, s in enumerate((0, 1)):
                tA[si * 64: si * 64 + 64] = C84[s, 0:64]
            for si, s in enumerate((2, 3)):
                tB[si * 64: si * 64 + 64] = C84[s, 0:64]
            for s in range(4):
                tC[s * 32: s * 32 + 20, 0:V] = C84[s, 64:84, 0:V]
            for tt in (tA, tB):
                tt[0:22, V:] = 0.0
                tt[64:86, V:] = 0.0
            Ctiles[i] = dict(A=tA, B=tB, C=tC)
        for pa in PLAN.passes:
            i = pa["dil"]
            V = SEQ_LEN - 2 * PADS[i]
            kind = pa["kind"]
            tcol = thr[:, pa["thr"]][:, None]
            if kind == "cntpair":
                TA = Ctiles[i]["A"]
                TB = Ctiles[i]["B"]
                v = (PAIR_W * (TA > tcol).sum(1, dtype=np.float64)
                     + (TB > tcol).sum(1, dtype=np.float64))
                accs["dve"][core, :, pa["acc"]] = v.astype(np.float32)
            elif kind == "cntpairC":
                TC = Ctiles[i]["C"][:, 0:V]
                t1col = thr[:, pa["thr1"]][:, None]
                v = (PAIR_W * (TC > tcol).sum(1, dtype=np.float64)
                     + (TC > t1col).sum(1, dtype=np.float64))
                accs["dve"][core, :, pa["acc"]] = v.astype(np.float32)
            else:
                w_ = pa["w"]
                T = Ctiles[i][pa["tile"]][:, 0:w_]
                if pa["eng"] == "dve":
                    if kind == "cnt":
                        v = (T > tcol).sum(1, dtype=np.float64)
                    else:
                        v = np.maximum(T, tcol).sum(1, dtype=np.float64)
                    accs["dve"][core, :, pa["acc"]] = v.astype(np.float32)
                else:
                    if kind == "cnt":
                        v = np.sign(T + tcol).sum(1, dtype=np.float64)
                    else:
                        v = np.maximum(T + tcol, 0).sum(1, dtype=np.float64)
                    accs["act"][core, :, pa["acc"]] = v.astype(np.float32)
    return accs
